# revision 36
# baseline (speedup 1.0000x reference)
"""LinOSS layer Trainium2 kernel (pipelined, merged r/i layout).

Math (same derivation as before): per-state recurrence collapses to
rank-2 modulated prefix sums
    u_t = s * Bu_t
    E = cumsum(T1 * u);  F = cumsum(T2 * u)
    x_t = sin(t th) * (E_t + oE) + cos(t th) * (F_t + oF)
    T1 = gamma*cos + sin;  T2 = cos - gamma*sin
with oE/oF the fold-chunk carry offsets.

Layout: 128 partitions = 2 fold-chunks x (32 real + 32 imag states);
free dim = 4096 time cols.  r/i share theta so one table row set serves
both; E/F merge the four scans of the old layout into two.

Pipeline: host sends input pre-transposed plus all four tables (no
on-chip table build, no DMA transpose).  Per 1024-col chunk: Bu matmuls
-> PSUM, DVE stt modulate straight from PSUM (accum_out collects row
sums for the carry), chained DVE scans (AP initial).  Carry offsets are
ready after the last modulate, so demod (ACT bias-add, DVE/Pool mults)
and projection matmuls overlap the remaining scans.
"""

import numpy as np

L, H, P = 8192, 128, 256
NCORES = 8
SLOC = P // NCORES          # 32 states per core
RI = 2 * SLOC               # 64 r+i rows per fold chunk
FOLD = 2
CL = L // FOLD              # 4096 free cols
SC = 1024                   # modulate/scan chunk
K = CL // SC                # 4
JT = 512                    # demod/project chunk
NJ = CL // JT               # 8

_CACHE: dict = {}


def _build_bass(split_waits=True):
    import concourse.bass as bass
    import concourse.mybir as mybir
    import concourse.tile as tile

    dt = mybir.dt.float32
    bt = mybir.dt.bfloat16
    Alu = mybir.AluOpType
    Ident = mybir.ActivationFunctionType.Identity

    nc = bass.Bass(
        trn_type="TRN2",
        target_bir_lowering=False,
        debug=False,
        num_devices=NCORES,
    )

    inpT_d = nc.dram_tensor("inpT", [H, L], bt, kind="ExternalInput").ap()
    Bt_d = nc.dram_tensor("Bt", [H, RI], bt, kind="ExternalInput").ap()
    Cx_d = nc.dram_tensor("Cx", [128, H], bt, kind="ExternalInput").ap()
    dD_d = nc.dram_tensor("dD", [H, H], bt, kind="ExternalInput").ap()
    Wm_d = nc.dram_tensor("Wm", [128, 128], dt, kind="ExternalInput").ap()
    T1_d = nc.dram_tensor("T1", [128, CL], bt, kind="ExternalInput").ap()
    T2_d = nc.dram_tensor("T2", [128, CL], bt, kind="ExternalInput").ap()
    sinT_d = nc.dram_tensor("sinT", [128, CL], bt, kind="ExternalInput").ap()
    cosT_d = nc.dram_tensor("cosT", [128, CL], bt, kind="ExternalInput").ap()
    outp = nc.dram_tensor("outp", [H, L], bt, kind="ExternalOutput").ap()

    with tile.TileContext(nc) as tc:
        cpool = tc.alloc_tile_pool(name="const", bufs=1)
        big = tc.alloc_tile_pool(name="big", bufs=1)
        work = tc.alloc_tile_pool(name="work", bufs=3)
        opool = tc.alloc_tile_pool(name="opool", bufs=6)
        pbu = tc.alloc_tile_pool(name="pbu", bufs=2, space="PSUM")
        pout = tc.alloc_tile_pool(name="pout", bufs=3, space="PSUM")
        poff = tc.alloc_tile_pool(name="poff", bufs=1, space="PSUM")

        Bt = cpool.tile([H, RI], bt)
        Cx = cpool.tile([128, H], bt)
        dD = cpool.tile([H, H], bt)
        Wm = cpool.tile([128, 128], dt)
        inpT = big.tile([H, L], bt, tag="inpT")
        T1 = big.tile([128, CL], bt, tag="T1")
        T2 = big.tile([128, CL], bt, tag="T2")
        sinT = big.tile([128, CL], bt, tag="sinT")
        cosT = big.tile([128, CL], bt, tag="cosT")
        E = big.tile([128, CL], bt, tag="E")
        F = big.tile([128, CL], bt, tag="F")

        NACC = K + 3            # k=0 runs as 4 sub-chunks
        ones_b = cpool.tile([128, SC], bt)
        zinit = cpool.tile([128, 1], dt)
        acc1 = cpool.tile([128, NACC], dt)
        acc2 = cpool.tile([128, NACC], dt)
        fins = cpool.tile([128, 2], dt)
        offs = cpool.tile([128, 2], dt)

        nc.gpsimd.memset(ones_b[:], 1.0)
        nc.gpsimd.memset(zinit[:], 0.0)

        # -- input / table DMAs --
        # One DMA's descriptors drain through a single HW queue (~25 GB/s),
        # so big transfers are split into pieces over four issue queues in
        # the order compute consumes them.  k=0 lands as 256-col pieces so
        # the first modulate starts ~10us in; k=2/3 stream on gpsimd; sin/
        # cos (phase 2 only) trail at the back.
        # k=0 criticals as 256-col pieces in strict need order on the two
        # HWDGE queues (sync/scalar); k=1 as 512-col pieces behind them.
        # k=2/3 + dD issue from inside the phase-1 loop on gpsimd so their
        # transfers spread out instead of fighting the early scans.
        nc.sync.dma_start(out=Bt[:], in_=Bt_d)
        for s in range(4):
            lo = s * 256
            nc.sync.dma_start(out=inpT[:, lo : lo + 256],
                              in_=inpT_d[:, lo : lo + 256])
            nc.scalar.dma_start(out=inpT[:, CL + lo : CL + lo + 256],
                                in_=inpT_d[:, CL + lo : CL + lo + 256])
            nc.sync.dma_start(out=T1[:, lo : lo + 256],
                              in_=T1_d[:, lo : lo + 256])
            nc.scalar.dma_start(out=T2[:, lo : lo + 256],
                                in_=T2_d[:, lo : lo + 256])
        nc.scalar.dma_start(out=Cx[:], in_=Cx_d)
        nc.sync.dma_start(out=Wm[:], in_=Wm_d)
        for s in range(2):
            lo = SC + s * 512
            nc.sync.dma_start(out=inpT[:, lo : lo + 512],
                              in_=inpT_d[:, lo : lo + 512])
            nc.scalar.dma_start(out=inpT[:, CL + lo : CL + lo + 512],
                                in_=inpT_d[:, CL + lo : CL + lo + 512])
            nc.sync.dma_start(out=T1[:, lo : lo + 512],
                              in_=T1_d[:, lo : lo + 512])
            nc.scalar.dma_start(out=T2[:, lo : lo + 512],
                                in_=T2_d[:, lo : lo + 512])
        for k in range(K):
            cs = slice(k * SC, (k + 1) * SC)
            nc.sync.dma_start(out=sinT[:, cs], in_=sinT_d[:, cs])
        nc.sync.dma_start(out=dD[:], in_=dD_d)
        # k2/k3 pieces + dD, doled out two per phase-1 chunk on gpsimd
        latep = []
        for k in range(2, K):
            for s in range(2):
                lo = k * SC + s * 512
                for c in range(FOLD):
                    latep.append((inpT, inpT_d, c * CL + lo, 512))
                latep.append((T1, T1_d, lo, 512))
                latep.append((T2, T2_d, lo, 512))

        # -- phase 1: Bu matmuls -> modulate -> chained scans --
        # k=0 split into 256-col sub-chunks so compute starts as soon as the
        # first DMA pieces land.
        chunks = [(s * 256, 256, s) for s in range(4)]
        chunks += [(k * SC, SC, 3 + k) for k in range(1, K)]
        for ci, (lo, w, ak) in enumerate(chunks):
            cs = slice(lo, lo + w)
            pb = pbu.tile([128, SC], dt, tag="bu")
            for h in range(max(1, w // 512)):
                mw = min(w, 512)
                hs = slice(h * 512, h * 512 + mw)
                for c in range(FOLD):
                    mc = c * CL + lo + h * 512
                    nc.tensor.matmul(
                        pb[c * RI : (c + 1) * RI, hs],
                        Bt[:], inpT[:, mc : mc + mw],
                        start=True, stop=True,
                        tile_position=(0, c * RI),
                    )
            Y1 = work.tile([128, SC], bt, tag="Y1")
            Y2 = work.tile([128, SC], bt, tag="Y2")
            usb = work.tile([128, SC], bt, tag="usb")
            # Y1 modulate on DVE (accum_out -> E totals); Y2 modulate on
            # Pool from an ACT-evac'd copy of u (Pool can't read PSUM).
            nc.scalar.copy(usb[:, 0:w], pb[:, 0:w])
            nc.vector.scalar_tensor_tensor(
                Y1[:, 0:w], pb[:, 0:w], 1.0, T1[:, cs], Alu.mult, Alu.mult,
                accum_out=acc1[:, ak : ak + 1],
            )
            nc.gpsimd.tensor_mul(Y2[:, 0:w], usb[:, 0:w], T2[:, cs])
            for _ in range(3):
                if latep:
                    dst, src, lo2, w2 = latep.pop(0)
                    nc.gpsimd.dma_start(out=dst[:, lo2 : lo2 + w2],
                                        in_=src[:, lo2 : lo2 + w2])
            if 3 <= ci <= 6:
                kc = slice((ci - 3) * SC, (ci - 2) * SC)
                nc.scalar.dma_start(out=cosT[:, kc], in_=cosT_d[:, kc])
            if ci == len(chunks) - 1:
                # E-offsets depend only on the modulate accums: hoist the
                # reduce ahead of the last scans so Wm/offs (PE+ACT) run
                # while DVE finishes scanning.
                nc.vector.tensor_reduce(
                    fins[:, 0:1], acc1[:], mybir.AxisListType.X, Alu.add)
            initE = zinit[:] if lo == 0 else E[:, lo - 1 : lo]
            initF = zinit[:] if lo == 0 else F[:, lo - 1 : lo]
            nc.vector.tensor_tensor_scan(
                E[:, cs], ones_b[:, 0:w], Y1[:, 0:w], initE, Alu.mult, Alu.add)
            nc.vector.tensor_tensor_scan(
                F[:, cs], ones_b[:, 0:w], Y2[:, 0:w], initF, Alu.mult, Alu.add)

        po = poff.tile([128, 2], dt, tag="off")
        nc.tensor.matmul(po[:, 0:1], Wm[:], fins[:, 0:1], start=True, stop=True)
        nc.scalar.copy(offs[:, 0:1], po[:, 0:1])
        # F totals = last column of the F scan (free), available at scan end
        nc.scalar.activation(fins[:, 1:2], F[:, CL - 1 : CL], Ident)
        nc.tensor.matmul(po[:, 1:2], Wm[:], fins[:, 1:2], start=True, stop=True)
        nc.scalar.copy(offs[:, 1:2], po[:, 1:2])

        # -- phase 2: demod (bias folded into DVE stts) + project + store --
        for j in range(NJ):
            jc = j * JT
            js = slice(jc, jc + JT)
            m1 = work.tile([128, JT], bt, tag="m1")
            m2 = work.tile([128, JT], bt, tag="m2")
            x0 = work.tile([128, JT], bt, tag="x0")
            nc.vector.scalar_tensor_tensor(
                m1[:], E[:, js], offs[:, 0:1], sinT[:, js], Alu.add, Alu.mult)
            nc.vector.scalar_tensor_tensor(
                m2[:], F[:, js], offs[:, 1:2], cosT[:, js], Alu.add, Alu.mult)
            nc.vector.tensor_add(x0[:], m1[:], m2[:])
            for c in range(FOLD):
                pc = pout.tile([128, JT], dt, tag="o")
                nc.tensor.matmul(
                    pc[:], Cx[c * RI : (c + 1) * RI, :],
                    x0[c * RI : (c + 1) * RI, :],
                    start=True, stop=False,
                    tile_position=(c * RI, 0),
                )
                nc.tensor.matmul(
                    pc[:], dD[:], inpT[:, c * CL + jc : c * CL + jc + JT],
                    start=False, stop=True,
                )
                osb = opool.tile([128, JT], bt, tag="osb")
                nc.scalar.copy(osb[:], pc[:])
                q = nc.sync if c == 0 else nc.scalar
                if j >= NJ - 2:
                    # split the tail DMAs so the final transfer drains fast
                    for u in range(2):
                        lo = c * CL + jc + u * 256
                        q2 = nc.sync if (c + u) % 2 == 0 else nc.scalar
                        q2.dma_start(out=outp[:, lo : lo + 256],
                                     in_=osb[:, u * 256 : (u + 1) * 256])
                else:
                    q.dma_start(
                        out=outp[:, c * CL + jc : c * CL + jc + JT], in_=osb[:])

        for p in (poff, pout, pbu, opool, work, big, cpool):
            p.release()
    if split_waits:
        _split_matmul_waits(nc, mybir)
    return nc


def _split_matmul_waits(nc, mybir):
    """Hardware instruction structs fit a limited number of embedded sync
    waits; move extra waits onto an inserted same-queue no-op."""
    caps = {"InstMatmult": 1}
    skip = {"InstNoOp", "InstAllEngineBarrier", "InstSync"}
    k = 0
    for bb in nc.main_func.blocks:
        insts = bb.instructions
        i = 0
        while i < len(insts):
            ins = insts[i]
            tn = type(ins).__name__
            if tn not in skip and ins.sync_info is not None:
                cap = caps.get(tn, 1)
                w = list(ins.sync_info.on_wait or [])
                if len(w) > cap:
                    for wj in w[:-cap]:
                        nop = mybir.InstNoOp(
                            name=f"I-mmdep-{k}",
                            engine=ins.engine,
                            ins=[],
                            outs=[],
                            sync_info=mybir.SyncInfo(
                                on_wait=[wj], on_update=[]
                            ),
                        )
                        k += 1
                        insts.insert(i, nop)
                        i += 1
                    ins.sync_info = mybir.SyncInfo(
                        on_wait=w[-cap:], on_update=ins.sync_info.on_update
                    )
            i += 1


def _host_prep(inputs):
    import ml_dtypes
    f32 = np.float32
    bf16 = ml_dtypes.bfloat16

    inpT = np.ascontiguousarray(
        np.asarray(inputs["input_sequence"], np.float32).T
    ).astype(bf16)
    A = np.maximum(np.asarray(inputs["A_diag_raw"], np.float64), 0.0)
    s = 1.0 / (1.0 + np.exp(-np.asarray(inputs["steps_raw"], np.float64)))
    Br = np.asarray(inputs["B_real"], np.float64)
    Bi = np.asarray(inputs["B_img"], np.float64)
    Cr = np.asarray(inputs["C_real"], np.float64)
    Ci = np.asarray(inputs["C_img"], np.float64)
    D = np.asarray(inputs["D"], np.float64)

    costh = 1.0 - s * s * A / 2.0
    sinth = np.sqrt(np.maximum(1.0 - costh * costh, 1e-300))
    theta = np.arctan2(sinth, costh)
    gamma = (s - s * s * A / 2.0) / sinth

    twopi = 2.0 * np.pi
    j = np.arange(CL, dtype=np.float64)
    in_maps = []
    for kcore in range(NCORES):
        sl = slice(kcore * SLOC, (kcore + 1) * SLOC)
        th_m = np.concatenate([theta[sl], theta[sl]])       # (RI,)
        g_m = np.concatenate([gamma[sl], gamma[sl]])        # (RI,)
        # partitions p = c*RI + m,  absolute time = c*CL + j
        ang = np.empty((128, CL), np.float64)
        for c in range(FOLD):
            tt = (c * CL + j)[None, :] * th_m[:, None]
            ang[c * RI : (c + 1) * RI] = np.mod(tt, twopi)
        sinT = np.sin(ang)
        cosT = np.cos(ang)
        g2 = np.tile(g_m, FOLD)[:, None]
        T1 = g2 * cosT + sinT
        T2 = cosT - g2 * sinT

        Bt = np.empty((H, RI), np.float64)
        Bt[:, 0:SLOC] = (s[sl, None] * Br[sl]).T
        Bt[:, SLOC:RI] = (s[sl, None] * Bi[sl]).T

        Cblk = np.concatenate([Cr[:, sl].T, -Ci[:, sl].T], axis=0)  # (RI, H)
        Cx = np.tile(Cblk, (FOLD, 1))                               # (128, H)

        dD = np.diag(D) if kcore == 0 else np.zeros((H, H))
        Wm = np.zeros((128, 128), f32)
        Wm[np.arange(RI), np.arange(RI) + RI] = 1.0

        in_maps.append({
            "inpT": inpT,
            "Bt": Bt.astype(bf16),
            "Cx": Cx.astype(bf16),
            "dD": dD.astype(bf16),
            "Wm": Wm,
            "T1": T1.astype(bf16),
            "T2": T2.astype(bf16),
            "sinT": sinT.astype(bf16),
            "cosT": cosT.astype(bf16),
        })
    return in_maps


LAST_RESULTS = None


def kernel(**inputs) -> np.ndarray:
    global LAST_RESULTS
    from concourse.bass_utils import run_bass_kernel_spmd

    if "nc" not in _CACHE:
        _CACHE["nc"] = _build_bass()
    nc = _CACHE["nc"]

    in_maps = _host_prep(inputs)
    res = run_bass_kernel_spmd(nc, in_maps, core_ids=list(range(NCORES)))
    LAST_RESULTS = res
    part = np.zeros((H, L), np.float32)
    for r in res.results:
        part += np.asarray(r["outp"], np.float32)
    return np.ascontiguousarray(part.T)
